# revision 7
# baseline (speedup 1.0000x reference)
"""Multi-head attention (B=2, S=2048, D=1024, H=16) on 8 Trainium2 cores.

Sharding: core c handles batch b = c//4 and head group g = c%4 (4 heads).
Output projection is row-sharded over head dims; per-core partial outputs
are summed on the host (bias added on the host).

Design (v2, ACT-bound): the scalar engine's exp is the hard floor
(131072 cols/head-group @ 1.2GHz + per-instr overhead ~= 146us), so the
schedule keeps ACT 100% busy on uniform [128,1024] exp tiles and hides all
PE work (QK deliberately NOT row-paired so PE duty stays ~93% and HAM stays
at 8/8) under it:

  per block t = (pair, i-super of 1024), step jb in 0..15:
    QK: S^T[j,i] = K_h^T x Q_h^T per head (serial, row-group 0)
    exp on ACT straight out of PSUM (N=1024)
    PV: block t-1 head1 during steps 0-7, own head0 during steps 8-15
        (V augmented with a ones column -> row 0 = softmax denominator)
    proj units + garbage filler matmuls keep PE duty over the HAM
    re-throttle threshold
  norm: reciprocal + gpsimd partition-broadcast + DVE multiply -> at_sb
  proj: y[i,mo] += A^T-chunk(stationary) x W^T(moving); bias on host
"""

import sys

sys.path.insert(0, "/opt/trn_rl_repo")

from contextlib import ExitStack

import numpy as np
import ml_dtypes

import concourse.bass as bass
import concourse.tile as tile
from concourse import bacc, mybir

N_CORES = 8
B, S, D_MODEL = 2, 2048, 1024
NUM_HEADS, D_K = 16, 64
H_PER_CORE = 4
SCALE = D_K ** -0.5
IS = 1024                 # i-super width
JB = S // 128             # 16 j-blocks
VA_W = 128                # ones col 0, zeros 1-63, v at 64-127
ET_BUFS = 44

F32 = mybir.dt.float32
BF16 = mybir.dt.bfloat16
AF = mybir.ActivationFunctionType
BLOCKS = [(0, 0), (1, 0), (0, 1024), (1, 1024)]  # (pair, i0), iw = 1024

# fold the last block's head-1 PV into steps 8-15 (else it runs in the tail)
FOLD_LAST = False


def ds(start, size):
    return slice(start, start + size)


def _trace(ctx: ExitStack, tc: tile.TileContext, io: dict):
    nc = tc.nc

    const = ctx.enter_context(tc.tile_pool(name="const", bufs=1))
    etp = ctx.enter_context(tc.tile_pool(name="et", bufs=ET_BUFS))
    normp = ctx.enter_context(tc.tile_pool(name="norm", bufs=2))
    atp = ctx.enter_context(tc.tile_pool(name="at", bufs=1))
    youtp = ctx.enter_context(tc.tile_pool(name="yout", bufs=2))
    miscp = ctx.enter_context(tc.tile_pool(name="misc", bufs=1))
    psS = ctx.enter_context(tc.tile_pool(name="psS", bufs=2, space="PSUM"))
    psO = ctx.enter_context(tc.tile_pool(name="psO", bufs=1, space="PSUM"))
    psY = ctx.enter_context(tc.tile_pool(name="psY", bufs=2, space="PSUM"))

    # ---- resident inputs ----
    # per-head [64, S] tiles at base partition 0 so every QK matmul sits in
    # PE row-group 0 (serial execution keeps PE duty high -> HAM stays 8/8)
    kt_sb = [const.tile([64, S], BF16, tag=f"kt{h}", name=f"kt{h}")
             for h in range(H_PER_CORE)]
    qt_sb = [const.tile([64, S], BF16, tag=f"qt{h}", name=f"qt{h}")
             for h in range(H_PER_CORE)]
    # head 0 slivers first so warmup + block-0 QK start early
    nc.sync.dma_start(kt_sb[0][:, 0:512], io["kt"][0][:, 0:512])
    nc.sync.dma_start(qt_sb[0][:, 0:512], io["qt"][0][:, 0:512])
    nc.sync.dma_start(kt_sb[0][:, 512:S], io["kt"][0][:, 512:S])
    nc.sync.dma_start(qt_sb[0][:, 512:S], io["qt"][0][:, 512:S])
    for h in range(1, H_PER_CORE):
        nc.sync.dma_start(kt_sb[h][:], io["kt"][h])
        nc.sync.dma_start(qt_sb[h][:], io["qt"][h])
    va_sb = const.tile([128, JB * H_PER_CORE * VA_W], BF16, tag="va")
    nc.sync.dma_start(va_sb[:], io["va"][:])
    wt_sb = []
    for p in range(2):
        t = const.tile([128, D_MODEL], BF16, tag=f"wt{p}")
        nc.sync.dma_start(t[:], io["wt"][p])
        wt_sb.append(t)
    at_sb = [atp.tile([128, S], BF16, tag=f"at{p}", name=f"at{p}")
             for p in range(2)]

    # ---- warmup ----
    # exp-table preload on ACT first (reads the first kt sliver directly so
    # the ~2.7us table load runs during input DMA), then ~10 matmuls to flip
    # HAM to 8/8 before block 0
    wexp = miscp.tile([1, 512], F32, tag="warm_exp", name="warm_exp")
    nc.scalar.activation(wexp[:], kt_sb[0][0:1, 0:512], AF.Exp, scale=SCALE)
    nc.sync.dma_start(io["wexp"][:], wexp[:])
    wps = psY.tile([128, 512], F32, tag="Y", name="warm_ps")
    for _ in range(10):
        nc.tensor.matmul(wps[:], kt_sb[0][:, 0:128], kt_sb[0][:, 0:512],
                         start=True, stop=True, skip_group_check=True)
    wsb = miscp.tile([1, 512], F32, tag="warm_out", name="warm_out")
    nc.vector.tensor_copy(wsb[:], wps[0:1, :])
    nc.sync.dma_start(io["warm"][:], wsb[:])

    ET = {}    # (block_idx, h2) -> list of 16 E tiles
    PSO = {}   # (block_idx, h2) -> psum tile(s)

    def emit_qk_exp(t, jb):
        pr, i0 = BLOCKS[t]
        for h2 in range(2):
            h = pr * 2 + h2
            sp = psS.tile([128, IS], F32, tag="S", name="sp")
            for nch in range(2):
                nc.tensor.matmul(
                    sp[:, ds(nch * 512, 512)],
                    kt_sb[h][:, ds(jb * 128, 128)],
                    qt_sb[h][:, ds(i0 + nch * 512, 512)],
                    start=True, stop=True,
                )
            e = etp.tile([128, IS], BF16, tag="et", name="e")
            nc.scalar.activation(e[:], sp[:], AF.Exp, scale=SCALE)
            ET[(t, h2)][jb] = e

    def emit_pv(t, h2, jbps, pool=None):
        pr, i0 = BLOCKS[t]
        h = pr * 2 + h2
        if (t, h2) not in PSO:
            if pool is None:
                PSO[(t, h2)] = [psO.tile([128, IS], F32, tag="O", name="psO")]
            else:
                PSO[(t, h2)] = [pool.tile([128, 512], F32, tag="Y",
                                          name=f"psOy{c}") for c in range(2)]
        tiles = PSO[(t, h2)]
        for jbp in jbps:
            for nch in range(2):
                if len(tiles) == 1:
                    out = tiles[0][0:128, ds(nch * 512, 512)]
                else:
                    out = tiles[nch][0:128, :]
                nc.tensor.matmul(
                    out,
                    va_sb[:, ds(jbp * H_PER_CORE * VA_W + h * VA_W, VA_W)],
                    ET[(t, h2)][jbp][:, ds(nch * 512, 512)],
                    start=(jbp == 0), stop=(jbp == JB - 1),
                    skip_group_check=True,
                )

    def emit_norm(t, h2, split=False):
        pr, i0 = BLOCKS[t]
        tiles = PSO[(t, h2)]
        if len(tiles) == 2:
            chunks = [(0, tiles[0], 0, 512), (1, tiles[1], 0, 512)]
        elif split:
            chunks = [(0, tiles[0], 0, 512), (1, tiles[0], 512, 512)]
        else:
            chunks = [(0, tiles[0], 0, IS)]
        for nch, O, off, w in chunks:
            rr = normp.tile([1, w], F32, tag="rr", name="rr")
            nc.vector.reciprocal_approx_fast(rr[:], O[0:1, ds(off, w)])
            bc = normp.tile([128, w], F32, tag="bc", name="bc")
            nc.gpsimd.partition_broadcast(bc[:], rr[0:1, :])
            nm = normp.tile([128, w], BF16, tag="nm", name="nm")
            nc.vector.tensor_mul(nm[64:128, :], O[64:128, ds(off, w)],
                                 bc[64:128, :])
            nc.sync.dma_start(
                at_sb[pr][ds(h2 * 64, 64), ds(i0 + nch * 512, w)],
                nm[64:128, :])
        del ET[(t, h2)]

    def emit_proj(unit, eng="vector"):
        ic, moch = unit
        Y = psY.tile([128, 512], F32, tag="Y")
        for hd2 in range(2):
            nc.tensor.matmul(
                Y[:],
                at_sb[hd2][:, ds(ic * 128, 128)],
                wt_sb[hd2][:, ds(moch * 512, 512)],
                start=(hd2 == 0), stop=(hd2 == 1),
                skip_group_check=True,
            )
        ysb = youtp.tile([128, 512], F32, tag="y")
        if eng == "vector":
            nc.vector.tensor_copy(ysb[:], Y[:])
        else:
            nc.scalar.copy(ysb[:], Y[:])
        nc.sync.dma_start(io["y"][ds(ic * 128, 128), ds(moch * 512, 512)],
                          ysb[:])

    def emit_fill(n, pool=psY, w=512):
        # garbage matmuls into an idle PSUM pool: keeps PE duty above the
        # HAM re-throttle threshold; results are never read
        for _ in range(n):
            f = pool.tile([128, w], F32, tag="Y", name="fill")
            nc.tensor.matmul(f[0:128, 0:512], kt_sb[0][:, 0:128],
                             qt_sb[0][:, 0:512],
                             start=True, stop=True, skip_group_check=True)

    # proj unit (ic, m) is ready once all 4 heads covering i-chunk ic are
    # normalized: ic 0-7 after norm(1,h1) at block-2 step 7; ic 8-15 in tail
    if FOLD_LAST:
        proj_sched = {
            (2, 1): [(ic, m) for ic in range(0, 8) for m in range(2)],
        }
    else:
        proj_sched = {
            (2, 1): [(ic, m) for ic in range(0, 4) for m in range(2)],
            (3, 0): [(ic, m) for ic in range(4, 6) for m in range(2)],
            (3, 1): [(ic, m) for ic in range(6, 8) for m in range(2)],
        }

    pops = 2 if FOLD_LAST else 1
    for t in range(len(BLOCKS)):
        for h2 in range(2):
            ET[(t, h2)] = [None] * JB
        projq0 = list(proj_sched.get((t, 0), []))
        projq1 = list(proj_sched.get((t, 1), []))
        for jb in range(JB):
            emit_qk_exp(t, jb)
            half = 0 if jb < 8 else 1
            projq = projq0 if half == 0 else projq1
            busy = 0
            if t == 0:
                # block 0 has no previous-block PV: own head-0 PV runs one
                # j-block per step (lag 1) and filler keeps the PE warm
                if 1 <= jb < JB - 1:
                    emit_pv(0, 0, [jb - 1])
                    busy += 1
                elif jb == JB - 1:
                    emit_pv(0, 0, [14, 15])
                    emit_norm(0, 0)
                    busy += 2
            else:
                if jb < 8:
                    emit_pv(t - 1, 1, [2 * jb, 2 * jb + 1])
                    busy += 2
                    if jb == 7:
                        emit_norm(t - 1, 1)
                else:
                    emit_pv(t, 0, [2 * (jb - 8), 2 * (jb - 8) + 1])
                    busy += 2
                    if jb == JB - 1:
                        emit_norm(t, 0)
                if FOLD_LAST and t == 3 and jb >= 8:
                    emit_pv(3, 1, [2 * (jb - 8), 2 * (jb - 8) + 1], pool=psY)
                    busy += 2
                    if jb == JB - 1:
                        emit_norm(3, 1)
            for _ in range(pops):
                if projq:
                    emit_proj(projq.pop(0))
                    busy += 1
            psy_reserved = FOLD_LAST and t == 3 and half == 1
            if busy < 3 and not projq and not psy_reserved:
                emit_fill(3 - busy)
        for u in projq0 + projq1:
            emit_proj(u)

    # tail: last block's head-1 PV (unless folded above), then the final
    # projection units for i-chunks 8-15
    if not FOLD_LAST:
        emit_pv(3, 1, list(range(JB)))
        emit_norm(3, 1, split=True)
    for k, (ic, m) in enumerate(((ic, m) for ic in range(8, 16)
                                 for m in range(2))):
        emit_proj((ic, m), eng=("scalar" if k % 2 else "vector"))


_CACHED_NC = None


def _build():
    global _CACHED_NC
    if _CACHED_NC is not None:
        return _CACHED_NC
    nc = bacc.Bacc("TRN2", target_bir_lowering=False, debug=False,
                   num_devices=N_CORES)
    io = {
        "qt": nc.dram_tensor("qt", [H_PER_CORE, 64, S], BF16,
                             kind="ExternalInput").ap(),
        "kt": nc.dram_tensor("kt", [H_PER_CORE, 64, S], BF16,
                             kind="ExternalInput").ap(),
        "va": nc.dram_tensor("va", [128, JB * H_PER_CORE * VA_W], BF16,
                             kind="ExternalInput").ap(),
        "wt": nc.dram_tensor("wt", [2, 128, D_MODEL], BF16,
                             kind="ExternalInput").ap(),
        "y": nc.dram_tensor("y", [S, D_MODEL], F32, kind="ExternalOutput").ap(),
        "warm": nc.dram_tensor("warm", [1, 512], F32,
                               kind="ExternalOutput").ap(),
        "wexp": nc.dram_tensor("wexp", [1, 512], F32,
                               kind="ExternalOutput").ap(),
    }
    with tile.TileContext(nc) as tc:
        with ExitStack() as ctx:
            _trace(ctx, tc, io)
    nc.compile()
    _CACHED_NC = nc
    return nc


def _core_inputs(q, k, v, W, b, core):
    bb, g = divmod(core, 4)
    hd0 = g * H_PER_CORE * D_K  # 256 per group
    ncol = H_PER_CORE * D_K
    bf = ml_dtypes.bfloat16

    qt = np.ascontiguousarray(q[bb, :, hd0:hd0 + ncol].T).reshape(4, 64, S)
    kt = np.ascontiguousarray(k[bb, :, hd0:hd0 + ncol].T).reshape(4, 64, S)
    v_sl = v[bb, :, hd0:hd0 + ncol].reshape(S, H_PER_CORE, D_K)
    va = np.concatenate(
        [np.ones((S, H_PER_CORE, 1), np.float32),
         np.zeros((S, H_PER_CORE, 63), np.float32), v_sl], axis=2
    ).reshape(JB, 128, H_PER_CORE * VA_W).transpose(1, 0, 2).reshape(
        128, JB * H_PER_CORE * VA_W)
    wt = np.ascontiguousarray(W[:, hd0:hd0 + ncol].T).reshape(2, 128, D_MODEL)
    return {
        "qt": qt.astype(bf),
        "kt": kt.astype(bf),
        "va": np.ascontiguousarray(va).astype(bf),
        "wt": wt.astype(bf),
    }


def run(inputs, trace=False, trace_kwargs=None):
    from concourse.bass_utils import run_bass_kernel_spmd

    q = np.asarray(inputs["q"], np.float32)
    k = np.asarray(inputs["k"], np.float32)
    v = np.asarray(inputs["v"], np.float32)
    W = np.asarray(inputs["W"], np.float32)
    b = np.asarray(inputs["b"], np.float32)

    nc = _build()
    in_maps = [_core_inputs(q, k, v, W, b, c) for c in range(N_CORES)]
    res = run_bass_kernel_spmd(nc, in_maps, core_ids=list(range(N_CORES)),
                               trace=trace, **(trace_kwargs or {}))
    out = np.empty((B, S, D_MODEL), np.float32)
    for bb in range(B):
        acc = res.results[bb * 4 + 0]["y"].astype(np.float32)
        for g in range(1, 4):
            acc = acc + res.results[bb * 4 + g]["y"]
        out[bb] = acc + b[None, :]
    return out, res


def kernel(**inputs):
    out, _ = run(inputs)
    return out


# revision 9
# speedup vs baseline: 1.0883x; 1.0883x over previous
"""Multi-head attention (B=2, S=2048, D=1024, H=16) on 8 Trainium2 cores.

Sharding: core c handles batch b = c//4 and head group g = c%4 (4 heads).
Output projection is row-sharded over head dims; per-core partial outputs
are summed on the host (bias added on the host).

Design (v3, ACT-bound): the scalar engine's exp is the hard floor
(131072 cols/core @ 1.2GHz + ~352cyc/instr overhead ~= 147us), so the
schedule keeps ACT 100% busy on uniform [128,1024] exp tiles and hides all
PE work under it. HAM (the PE clock gate) un-throttles only when the MAC
array is nearly saturated per 3.4us window, so:
  - QK keeps the baseline's two-head row-group pairing (both PE halves
    compute concurrently -> full-array activity),
  - full-array garbage "fill" matmuls (K=128, 512 cols) top every step up
    to ~2.1us of MAC time so HAM stays at 8/8 for the whole run.

Per block t = (pair, i-super of 1024), step jb in 0..15:
    QK: S^T[j,i] = K_h^T x Q_h^T, two heads interleaved (concurrent tiles)
    exp on ACT straight out of PSUM (N=1024)
    PV: prev-block head1 on steps 2..7, own head0 on steps 10..15
        (start offsets clear the preceding norm chain's psO occupancy;
        V augmented with a ones column -> PSUM row 0 = softmax denominator)
    proj units once all 4 heads of an i-chunk are normalized
norm: DVE reciprocal + gpsimd partition-broadcast + DVE multiply -> at_sb
proj: y[i,mo] += A^T-chunk(stationary) x W^T(moving); bias on host.
"""

import sys

sys.path.insert(0, "/opt/trn_rl_repo")

from contextlib import ExitStack

import numpy as np
import ml_dtypes

import concourse.bass as bass
import concourse.tile as tile
from concourse import bacc, mybir

N_CORES = 8
B, S, D_MODEL = 2, 2048, 1024
NUM_HEADS, D_K = 16, 64
H_PER_CORE = 4
SCALE = D_K ** -0.5
IS = 1024                 # i-super width
JB = S // 128             # 16 j-blocks
VA_W = 128                # ones col 0, zeros 1-63, v at 64-127
VA_CHUNK = 4              # va split into 4 tiles of 4 j-blocks each
ET_BUFS = 44

F32 = mybir.dt.float32
BF16 = mybir.dt.bfloat16
AF = mybir.ActivationFunctionType
BLOCKS = [(0, 0), (1, 0), (0, 1024), (1, 1024)]  # (pair, i0), iw = 1024

MAC_TARGET = 2100         # ns of full-array MAC time to emit per step
FILL_MAC = 213

# per-step jbp batches: 16 j-blocks over steps 2..7 (head1 of prev block)
PV1_SCHED = {2: [0, 1, 2], 3: [3, 4, 5], 4: [6, 7, 8], 5: [9, 10, 11],
             6: [12, 13], 7: [14, 15]}
# ... and over steps 10..15 (own head0)
PV0_SCHED = {10: [0, 1, 2], 11: [3, 4, 5], 12: [6, 7, 8], 13: [9, 10, 11],
             14: [12, 13], 15: [14, 15]}


def ds(start, size):
    return slice(start, start + size)


def _trace(ctx: ExitStack, tc: tile.TileContext, io: dict):
    nc = tc.nc

    const = ctx.enter_context(tc.tile_pool(name="const", bufs=1))
    etp = ctx.enter_context(tc.tile_pool(name="et", bufs=ET_BUFS))
    normp = ctx.enter_context(tc.tile_pool(name="norm", bufs=2))
    atp = ctx.enter_context(tc.tile_pool(name="at", bufs=1))
    youtp = ctx.enter_context(tc.tile_pool(name="yout", bufs=2))
    miscp = ctx.enter_context(tc.tile_pool(name="misc", bufs=2))
    psS = ctx.enter_context(tc.tile_pool(name="psS", bufs=2, space="PSUM"))
    psO = ctx.enter_context(tc.tile_pool(name="psO", bufs=1, space="PSUM"))
    psY = ctx.enter_context(tc.tile_pool(name="psY", bufs=2, space="PSUM"))

    # ---- resident inputs (order matters: earliest-needed first) ----
    kt_sb = [const.tile([128, S], BF16, tag=f"kt{p}", name=f"kt{p}")
             for p in range(2)]
    qt_sb = [const.tile([128, S], BF16, tag=f"qt{p}", name=f"qt{p}")
             for p in range(2)]
    va_sb = [const.tile([128, JB // VA_CHUNK * H_PER_CORE * VA_W], BF16,
                        tag=f"va{c}", name=f"va{c}") for c in range(VA_CHUNK)]
    nc.sync.dma_start(kt_sb[0][:, 0:512], io["kt"][0][:, 0:512])
    nc.sync.dma_start(qt_sb[0][:, 0:IS], io["qt"][0][:, 0:IS])
    nc.sync.dma_start(va_sb[0][:], io["va"][0])
    nc.sync.dma_start(kt_sb[0][:, 512:S], io["kt"][0][:, 512:S])
    nc.sync.dma_start(va_sb[1][:], io["va"][1])
    nc.sync.dma_start(kt_sb[1][:], io["kt"][1])
    nc.sync.dma_start(qt_sb[1][:, 0:IS], io["qt"][1][:, 0:IS])
    nc.sync.dma_start(va_sb[2][:], io["va"][2])
    nc.sync.dma_start(va_sb[3][:], io["va"][3])
    wt_sb = []
    for p in range(2):
        t = const.tile([128, D_MODEL], BF16, tag=f"wt{p}")
        nc.sync.dma_start(t[:], io["wt"][p])
        wt_sb.append(t)
    nc.sync.dma_start(qt_sb[0][:, IS:S], io["qt"][0][:, IS:S])
    nc.sync.dma_start(qt_sb[1][:, IS:S], io["qt"][1][:, IS:S])
    at_sb = [atp.tile([128, S], BF16, tag=f"at{p}", name=f"at{p}")
             for p in range(2)]

    # ---- warmup ----
    # exp-table preload on ACT (reads the first kt sliver, so the ~2.7us
    # table load overlaps input DMA), then 10 full-array matmuls for HAM
    wexp = miscp.tile([1, 512], F32, tag="warm_exp", name="warm_exp")
    nc.scalar.activation(wexp[:], kt_sb[0][0:1, 0:512], AF.Exp, scale=SCALE)
    nc.sync.dma_start(io["wexp"][:], wexp[:])
    wps = psY.tile([128, 512], F32, tag="Y", name="warm_ps")
    for _ in range(10):
        nc.tensor.matmul(wps[:], kt_sb[0][:, 0:128], kt_sb[0][:, 0:512],
                         start=True, stop=True, skip_group_check=True)
    wsb = miscp.tile([1, 512], F32, tag="warm_out", name="warm_out")
    nc.vector.tensor_copy(wsb[:], wps[0:1, :])
    nc.sync.dma_start(io["warm"][:], wsb[:])

    ET = {}    # (block_idx, h2) -> list of 16 E tiles
    PSO = {}   # (block_idx, h2) -> psum tile

    def emit_qk_exp(t, jb):
        pr, i0 = BLOCKS[t]
        # interleave the two heads' matmuls: distinct PE row-groups run
        # concurrently (full-array MAC activity keeps HAM at 8/8)
        sps = [psS.tile([128, IS], F32, tag="S", name="sp") for _ in range(2)]
        for nch in range(2):
            for h2 in range(2):
                nc.tensor.matmul(
                    sps[h2][:, ds(nch * 512, 512)],
                    kt_sb[pr][ds(h2 * 64, 64), ds(jb * 128, 128)],
                    qt_sb[pr][ds(h2 * 64, 64), ds(i0 + nch * 512, 512)],
                    start=True, stop=True,
                )
        for h2 in range(2):
            e = etp.tile([128, IS], BF16, tag="et", name="e")
            nc.scalar.activation(e[:], sps[h2][:], AF.Exp, scale=SCALE)
            ET[(t, h2)][jb] = e

    def emit_pv(t, h2, jbps):
        pr, i0 = BLOCKS[t]
        h = pr * 2 + h2
        if (t, h2) not in PSO:
            PSO[(t, h2)] = psO.tile([128, IS], F32, tag="O", name="psO")
        O = PSO[(t, h2)]
        for jbp in jbps:
            va = va_sb[jbp // VA_CHUNK]
            vo = (jbp % VA_CHUNK) * H_PER_CORE * VA_W + h * VA_W
            for nch in range(2):
                nc.tensor.matmul(
                    O[0:128, ds(nch * 512, 512)],
                    va[:, ds(vo, VA_W)],
                    ET[(t, h2)][jbp][:, ds(nch * 512, 512)],
                    start=(jbp == 0), stop=(jbp == JB - 1),
                    skip_group_check=True,
                )

    def emit_norm(t, h2, split=False):
        pr, i0 = BLOCKS[t]
        O = PSO[(t, h2)]
        chunks = [(0, 512), (512, 512)] if split else [(0, IS)]
        for off, w in chunks:
            rr = normp.tile([1, w], F32, tag="rr", name="rr")
            nc.vector.reciprocal_approx_fast(rr[:], O[0:1, ds(off, w)])
            bc = normp.tile([128, w], F32, tag="bc", name="bc")
            nc.gpsimd.partition_broadcast(bc[:], rr[0:1, :])
            nm = normp.tile([128, w], BF16, tag="nm", name="nm")
            nc.vector.tensor_mul(nm[64:128, :], O[64:128, ds(off, w)],
                                 bc[64:128, :])
            nc.sync.dma_start(
                at_sb[pr][ds(h2 * 64, 64), ds(i0 + off, w)],
                nm[64:128, :])
        del ET[(t, h2)]

    def emit_proj(unit, eng="vector"):
        ic, moch = unit
        Y = psY.tile([128, 512], F32, tag="Y")
        for hd2 in range(2):
            nc.tensor.matmul(
                Y[:],
                at_sb[hd2][:, ds(ic * 128, 128)],
                wt_sb[hd2][:, ds(moch * 512, 512)],
                start=(hd2 == 0), stop=(hd2 == 1),
                skip_group_check=True,
            )
        ysb = youtp.tile([128, 512], F32, tag="y")
        if eng == "vector":
            nc.vector.tensor_copy(ysb[:], Y[:])
        else:
            nc.scalar.copy(ysb[:], Y[:])
        nc.sync.dma_start(io["y"][ds(ic * 128, 128), ds(moch * 512, 512)],
                          ysb[:])

    def emit_fill(mac_ns):
        # full-array garbage matmuls (K=128, 512 cols): keep the PE's MAC
        # duty above the HAM re-throttle threshold; results are never read
        n = max(0, round(mac_ns / FILL_MAC))
        for _ in range(n):
            f = psY.tile([128, 512], F32, tag="Y", name="fill")
            nc.tensor.matmul(f[:], wt_sb[0][:, 0:128], wt_sb[1][:, 0:512],
                             start=True, stop=True, skip_group_check=True)

    # proj unit (ic, m) is ready once all 4 heads covering i-chunk ic are
    # normalized: ic 0-7 after norm(1,h1) early in block 2; ic 8-15 in tail
    proj_sched = {
        (2, 1): [(ic, m) for ic in range(0, 4) for m in range(2)],
        (3, 0): [(ic, m) for ic in range(4, 6) for m in range(2)],
        (3, 1): [(ic, m) for ic in range(6, 8) for m in range(2)],
    }

    for t in range(len(BLOCKS)):
        for h2 in range(2):
            ET[(t, h2)] = [None] * JB
        projq0 = list(proj_sched.get((t, 0), []))
        projq1 = list(proj_sched.get((t, 1), []))
        for jb in range(JB):
            emit_qk_exp(t, jb)
            mac = 426
            projq = projq0 if jb < 8 else projq1
            if t == 0:
                # no previous-block PV: spread own head0 one j-block per step
                if 1 <= jb < JB - 1:
                    emit_pv(0, 0, [jb - 1])
                    mac += 426
                elif jb == JB - 1:
                    emit_pv(0, 0, [14, 15])
                    emit_norm(0, 0)
                    mac += 852
            else:
                if jb in PV1_SCHED:
                    jbps = PV1_SCHED[jb]
                    emit_pv(t - 1, 1, jbps)
                    mac += 426 * len(jbps)
                    if jb == 7:
                        emit_norm(t - 1, 1)
                if jb in PV0_SCHED:
                    jbps = PV0_SCHED[jb]
                    emit_pv(t, 0, jbps)
                    mac += 426 * len(jbps)
                    if jb == JB - 1:
                        emit_norm(t, 0)
            if projq:
                emit_proj(projq.pop(0))
                mac += 426
            emit_fill(MAC_TARGET - mac)
        for u in projq0 + projq1:
            emit_proj(u)

    # tail: last block's head-1 PV, split norm, final proj for i-chunks 8-15
    emit_fill(4000)  # psO is freed only after norm(3,0)'s engine chain
    emit_pv(3, 1, list(range(JB)))
    emit_norm(3, 1, split=True)
    emit_fill(2400)  # cover the norm chain so HAM stays warm into the tail
    for k, (ic, m) in enumerate(((ic, m) for ic in range(8, 16)
                                 for m in range(2))):
        emit_proj((ic, m), eng=("scalar" if k % 2 else "vector"))
        emit_fill(213)


_CACHED_NC = None


def _build():
    global _CACHED_NC
    if _CACHED_NC is not None:
        return _CACHED_NC
    nc = bacc.Bacc("TRN2", target_bir_lowering=False, debug=False,
                   num_devices=N_CORES)
    va_cols = JB // VA_CHUNK * H_PER_CORE * VA_W
    io = {
        "qt": nc.dram_tensor("qt", [2, 128, S], BF16,
                             kind="ExternalInput").ap(),
        "kt": nc.dram_tensor("kt", [2, 128, S], BF16,
                             kind="ExternalInput").ap(),
        "va": nc.dram_tensor("va", [VA_CHUNK, 128, va_cols], BF16,
                             kind="ExternalInput").ap(),
        "wt": nc.dram_tensor("wt", [2, 128, D_MODEL], BF16,
                             kind="ExternalInput").ap(),
        "y": nc.dram_tensor("y", [S, D_MODEL], F32, kind="ExternalOutput").ap(),
        "warm": nc.dram_tensor("warm", [1, 512], F32,
                               kind="ExternalOutput").ap(),
        "wexp": nc.dram_tensor("wexp", [1, 512], F32,
                               kind="ExternalOutput").ap(),
    }
    with tile.TileContext(nc) as tc:
        with ExitStack() as ctx:
            _trace(ctx, tc, io)
    nc.compile()
    _CACHED_NC = nc
    return nc


def _core_inputs(q, k, v, W, b, core):
    bb, g = divmod(core, 4)
    hd0 = g * H_PER_CORE * D_K  # 256 per group
    ncol = H_PER_CORE * D_K
    bf = ml_dtypes.bfloat16

    qt = np.ascontiguousarray(q[bb, :, hd0:hd0 + ncol].T).reshape(2, 128, S)
    kt = np.ascontiguousarray(k[bb, :, hd0:hd0 + ncol].T).reshape(2, 128, S)
    v_sl = v[bb, :, hd0:hd0 + ncol].reshape(S, H_PER_CORE, D_K)
    va = np.concatenate(
        [np.ones((S, H_PER_CORE, 1), np.float32),
         np.zeros((S, H_PER_CORE, 63), np.float32), v_sl], axis=2
    ).reshape(JB, 128, H_PER_CORE * VA_W).transpose(1, 0, 2).reshape(
        128, JB * H_PER_CORE * VA_W)
    va = va.reshape(128, VA_CHUNK, JB // VA_CHUNK * H_PER_CORE * VA_W)
    va = np.ascontiguousarray(va.transpose(1, 0, 2))
    wt = np.ascontiguousarray(W[:, hd0:hd0 + ncol].T).reshape(2, 128, D_MODEL)
    return {
        "qt": qt.astype(bf),
        "kt": kt.astype(bf),
        "va": va.astype(bf),
        "wt": wt.astype(bf),
    }


def run(inputs, trace=False, trace_kwargs=None):
    from concourse.bass_utils import run_bass_kernel_spmd

    q = np.asarray(inputs["q"], np.float32)
    k = np.asarray(inputs["k"], np.float32)
    v = np.asarray(inputs["v"], np.float32)
    W = np.asarray(inputs["W"], np.float32)
    b = np.asarray(inputs["b"], np.float32)

    nc = _build()
    in_maps = [_core_inputs(q, k, v, W, b, c) for c in range(N_CORES)]
    res = run_bass_kernel_spmd(nc, in_maps, core_ids=list(range(N_CORES)),
                               trace=trace, **(trace_kwargs or {}))
    out = np.empty((B, S, D_MODEL), np.float32)
    for bb in range(B):
        acc = res.results[bb * 4 + 0]["y"].astype(np.float32)
        for g in range(1, 4):
            acc = acc + res.results[bb * 4 + g]["y"]
        out[bb] = acc + b[None, :]
    return out, res


def kernel(**inputs):
    out, _ = run(inputs)
    return out


# revision 14
# speedup vs baseline: 1.1791x; 1.0834x over previous
"""Multi-head attention (B=2, S=2048, D=1024, H=16) on 8 Trainium2 cores.

Sharding: core c handles batch b = c//4 and head group g = c%4 (4 heads).
Output projection is row-sharded over head dims; per-core partial outputs
are summed on the host (bias added on the host).

Design (v3, ACT-bound): the scalar engine's exp is the hard floor
(131072 cols/core @ 1.2GHz + ~352cyc/instr overhead ~= 147us), so the
schedule keeps ACT 100% busy on uniform [128,1024] exp tiles and hides all
PE work under it. HAM (the PE clock gate) un-throttles only when the MAC
array is nearly saturated per 3.4us window, so:
  - QK keeps the baseline's two-head row-group pairing (both PE halves
    compute concurrently -> full-array activity),
  - full-array garbage "fill" matmuls (K=128, 512 cols) top every step up
    to ~2.1us of MAC time so HAM stays at 8/8 for the whole run.

Per block t = (pair, i-super of 1024), step jb in 0..15:
    QK: S^T[j,i] = K_h^T x Q_h^T, two heads interleaved (concurrent tiles)
    exp on ACT straight out of PSUM (N=1024)
    PV: prev-block head1 on steps 2..7, own head0 on steps 10..15
        (start offsets clear the preceding norm chain's psO occupancy;
        V augmented with a ones column -> PSUM row 0 = softmax denominator)
    proj units once all 4 heads of an i-chunk are normalized
norm: DVE reciprocal + gpsimd partition-broadcast + DVE multiply -> at_sb
proj: y[i,mo] += A^T-chunk(stationary) x W^T(moving); bias on host.
"""

import sys

sys.path.insert(0, "/opt/trn_rl_repo")

from contextlib import ExitStack

import numpy as np
import ml_dtypes

import concourse.bass as bass
import concourse.tile as tile
from concourse import bacc, mybir

N_CORES = 8
B, S, D_MODEL = 2, 2048, 1024
NUM_HEADS, D_K = 16, 64
H_PER_CORE = 4
SCALE = D_K ** -0.5
IS = 1024                 # i-super width
JB = S // 128             # 16 j-blocks
VA_W = 128                # ones col 0, zeros 1-63, v at 64-127
VA_CHUNK = 4              # va split into 4 tiles of 4 j-blocks each
ET_BUFS = 44

F32 = mybir.dt.float32
BF16 = mybir.dt.bfloat16
AF = mybir.ActivationFunctionType
BLOCKS = [(0, 0), (1, 0), (0, 1024), (1, 1024)]  # (pair, i0), iw = 1024

MAC_TARGET = 2100         # ns of full-array MAC time to emit per step
FILL_MAC = 213

# per-step jbp batches: 16 j-blocks over steps 2..7 (head1 of prev block)
PV1_SCHED = {2: [0, 1, 2], 3: [3, 4, 5], 4: [6, 7, 8], 5: [9, 10, 11],
             6: [12, 13], 7: [14, 15]}
# ... and over steps 10..15 (own head0)
PV0_SCHED = {10: [0, 1, 2], 11: [3, 4, 5], 12: [6, 7, 8], 13: [9, 10, 11],
             14: [12, 13], 15: [14, 15]}


def ds(start, size):
    return slice(start, start + size)


def _trace(ctx: ExitStack, tc: tile.TileContext, io: dict):
    nc = tc.nc

    const = ctx.enter_context(tc.tile_pool(name="const", bufs=1))
    etp = ctx.enter_context(tc.tile_pool(name="et", bufs=ET_BUFS))
    normp = ctx.enter_context(tc.tile_pool(name="norm", bufs=2))
    atp = ctx.enter_context(tc.tile_pool(name="at", bufs=1))
    youtp = ctx.enter_context(tc.tile_pool(name="yout", bufs=6))
    miscp = ctx.enter_context(tc.tile_pool(name="misc", bufs=2))
    psS = ctx.enter_context(tc.tile_pool(name="psS", bufs=2, space="PSUM"))
    psO = ctx.enter_context(tc.tile_pool(name="psO", bufs=1, space="PSUM"))
    psY = ctx.enter_context(tc.tile_pool(name="psY", bufs=2, space="PSUM"))

    # ---- resident inputs (order matters: earliest-needed first) ----
    kt_sb = [const.tile([128, S], BF16, tag=f"kt{p}", name=f"kt{p}")
             for p in range(2)]
    qt_sb = [const.tile([128, S], BF16, tag=f"qt{p}", name=f"qt{p}")
             for p in range(2)]
    va_sb = [const.tile([128, JB // VA_CHUNK * H_PER_CORE * VA_W], BF16,
                        tag=f"va{c}", name=f"va{c}") for c in range(VA_CHUNK)]
    nc.sync.dma_start(kt_sb[0][:, 0:512], io["kt"][0][:, 0:512])
    nc.sync.dma_start(qt_sb[0][:, 0:IS], io["qt"][0][:, 0:IS])
    nc.sync.dma_start(va_sb[0][:], io["va"][0])
    nc.sync.dma_start(kt_sb[0][:, 512:S], io["kt"][0][:, 512:S])
    nc.sync.dma_start(va_sb[1][:], io["va"][1])
    nc.sync.dma_start(kt_sb[1][:], io["kt"][1])
    nc.sync.dma_start(qt_sb[1][:, 0:IS], io["qt"][1][:, 0:IS])
    nc.sync.dma_start(va_sb[2][:], io["va"][2])
    nc.sync.dma_start(va_sb[3][:], io["va"][3])
    wt_sb = []
    for p in range(2):
        t = const.tile([128, D_MODEL], BF16, tag=f"wt{p}")
        nc.sync.dma_start(t[:], io["wt"][p])
        wt_sb.append(t)
    nc.sync.dma_start(qt_sb[0][:, IS:S], io["qt"][0][:, IS:S])
    nc.sync.dma_start(qt_sb[1][:, IS:S], io["qt"][1][:, IS:S])
    at_sb = [atp.tile([128, S], BF16, tag=f"at{p}", name=f"at{p}")
             for p in range(2)]

    # ---- warmup ----
    # exp-table preload on ACT (reads the first kt sliver, so the ~2.7us
    # table load overlaps input DMA), then 10 full-array matmuls for HAM
    wexp = miscp.tile([1, 512], F32, tag="warm_exp", name="warm_exp")
    nc.scalar.activation(wexp[:], kt_sb[0][0:1, 0:512], AF.Exp, scale=SCALE)
    nc.sync.dma_start(io["wexp"][:], wexp[:])
    wps = psY.tile([128, 512], F32, tag="Y", name="warm_ps")
    for _ in range(4):
        nc.tensor.matmul(wps[:], kt_sb[0][:, 0:128], kt_sb[0][:, 0:512],
                         start=True, stop=True, skip_group_check=True)
    wsb = miscp.tile([1, 512], F32, tag="warm_out", name="warm_out")
    nc.vector.tensor_copy(wsb[:], wps[0:1, :])
    nc.sync.dma_start(io["warm"][:], wsb[:])

    ET = {}    # (block_idx, h2) -> list of 16 E tiles
    PSO = {}   # (block_idx, h2) -> psum tile

    def emit_qk_exp(t, jb):
        pr, i0 = BLOCKS[t]
        # interleave the two heads' matmuls: distinct PE row-groups run
        # concurrently (full-array MAC activity keeps HAM at 8/8)
        sps = [psS.tile([128, IS], F32, tag="S", name="sp") for _ in range(2)]
        for nch in range(2):
            for h2 in range(2):
                nc.tensor.matmul(
                    sps[h2][:, ds(nch * 512, 512)],
                    kt_sb[pr][ds(h2 * 64, 64), ds(jb * 128, 128)],
                    qt_sb[pr][ds(h2 * 64, 64), ds(i0 + nch * 512, 512)],
                    start=True, stop=True,
                )
        for h2 in range(2):
            e = etp.tile([128, IS], BF16, tag="et", name="e")
            nc.scalar.activation(e[:], sps[h2][:], AF.Exp, scale=SCALE)
            ET[(t, h2)][jb] = e

    def emit_pv(t, h2, jbps):
        pr, i0 = BLOCKS[t]
        h = pr * 2 + h2
        if (t, h2) not in PSO:
            PSO[(t, h2)] = psO.tile([128, IS], F32, tag="O", name="psO")
        O = PSO[(t, h2)]
        for jbp in jbps:
            va = va_sb[jbp // VA_CHUNK]
            vo = (jbp % VA_CHUNK) * H_PER_CORE * VA_W + h * VA_W
            for nch in range(2):
                nc.tensor.matmul(
                    O[0:128, ds(nch * 512, 512)],
                    va[:, ds(vo, VA_W)],
                    ET[(t, h2)][jbp][:, ds(nch * 512, 512)],
                    start=(jbp == 0), stop=(jbp == JB - 1),
                    skip_group_check=True,
                )

    def emit_norm(t, h2, split=False):
        pr, i0 = BLOCKS[t]
        O = PSO[(t, h2)]
        chunks = [(0, 512), (512, 512)] if split else [(0, IS)]
        for off, w in chunks:
            rr = normp.tile([1, w], F32, tag="rr", name="rr")
            nc.vector.reciprocal_approx_fast(rr[:], O[0:1, ds(off, w)])
            bc = normp.tile([128, w], F32, tag="bc", name="bc")
            nc.gpsimd.partition_broadcast(bc[:], rr[0:1, :])
            nm = normp.tile([128, w], BF16, tag="nm", name="nm")
            nc.vector.tensor_mul(nm[64:128, :], O[64:128, ds(off, w)],
                                 bc[64:128, :])
            nc.sync.dma_start(
                at_sb[pr][ds(h2 * 64, 64), ds(i0 + off, w)],
                nm[64:128, :])
        del ET[(t, h2)]

    def emit_proj(unit, eng="vector"):
        ic, moch = unit
        Y = psY.tile([128, 512], F32, tag="Y")
        for hd2 in range(2):
            nc.tensor.matmul(
                Y[:],
                at_sb[hd2][:, ds(ic * 128, 128)],
                wt_sb[hd2][:, ds(moch * 512, 512)],
                start=(hd2 == 0), stop=(hd2 == 1),
                skip_group_check=True,
            )
        ysb = youtp.tile([128, 512], BF16, tag="y")
        if eng == "vector":
            nc.vector.tensor_copy(ysb[:], Y[:])
        else:
            nc.scalar.copy(ysb[:], Y[:])
        nc.sync.dma_start(io["y"][ds(ic * 128, 128), ds(moch * 512, 512)],
                          ysb[:])

    def emit_fill(mac_ns):
        # full-array garbage matmuls (K=128, 512 cols): keep the PE's MAC
        # duty above the HAM re-throttle threshold; results are never read
        n = max(0, round(mac_ns / FILL_MAC))
        for _ in range(n):
            f = psY.tile([128, 512], F32, tag="Y", name="fill")
            nc.tensor.matmul(f[:], kt_sb[0][:, 0:128], kt_sb[0][:, 0:512],
                             start=True, stop=True, skip_group_check=True)

    # proj unit (ic, m) is ready once all 4 heads covering i-chunk ic are
    # normalized: ic 0-7 after norm(1,h1) early in block 2; ic 8-15 in tail
    proj_sched = {
        (2, 1): [(ic, m) for ic in range(0, 4) for m in range(2)],
        (3, 0): [(ic, m) for ic in range(4, 6) for m in range(2)],
        (3, 1): [(ic, m) for ic in range(6, 8) for m in range(2)],
    }

    for t in range(len(BLOCKS)):
        for h2 in range(2):
            ET[(t, h2)] = [None] * JB
        projq0 = list(proj_sched.get((t, 0), []))
        projq1 = list(proj_sched.get((t, 1), []))
        for jb in range(JB):
            emit_qk_exp(t, jb)
            mac = 426
            projq = projq0 if jb < 8 else projq1
            if t == 0:
                # no previous-block PV: spread own head0 one j-block per step
                if 1 <= jb < JB - 1:
                    emit_pv(0, 0, [jb - 1])
                    mac += 426
                elif jb == JB - 1:
                    emit_pv(0, 0, [14, 15])
                    emit_norm(0, 0)
                    mac += 852
            else:
                if jb in PV1_SCHED:
                    jbps = PV1_SCHED[jb]
                    emit_pv(t - 1, 1, jbps)
                    mac += 426 * len(jbps)
                    if jb == 7:
                        emit_norm(t - 1, 1)
                if jb in PV0_SCHED:
                    jbps = PV0_SCHED[jb]
                    emit_pv(t, 0, jbps)
                    mac += 426 * len(jbps)
                    if jb == JB - 1:
                        emit_norm(t, 0)
            if projq:
                emit_proj(projq.pop(0))
                mac += 426
            emit_fill(MAC_TARGET - mac)
        for u in projq0 + projq1:
            emit_proj(u)

    # tail: last block's head-1 PV, split norm, final proj for i-chunks 8-15
    emit_fill(4000)  # psO is freed only after norm(3,0)'s engine chain
    emit_pv(3, 1, list(range(JB)))
    emit_norm(3, 1, split=True)
    emit_fill(2400)  # cover the norm chain so HAM stays warm into the tail
    for k, (ic, m) in enumerate(((ic, m) for ic in range(8, 16)
                                 for m in range(2))):
        emit_proj((ic, m), eng=("scalar" if k % 2 else "vector"))
        emit_fill(213)


_CACHED_NC = None


def _build():
    global _CACHED_NC
    if _CACHED_NC is not None:
        return _CACHED_NC
    nc = bacc.Bacc("TRN2", target_bir_lowering=False, debug=False,
                   num_devices=N_CORES)
    va_cols = JB // VA_CHUNK * H_PER_CORE * VA_W
    io = {
        "qt": nc.dram_tensor("qt", [2, 128, S], BF16,
                             kind="ExternalInput").ap(),
        "kt": nc.dram_tensor("kt", [2, 128, S], BF16,
                             kind="ExternalInput").ap(),
        "va": nc.dram_tensor("va", [VA_CHUNK, 128, va_cols], BF16,
                             kind="ExternalInput").ap(),
        "wt": nc.dram_tensor("wt", [2, 128, D_MODEL], BF16,
                             kind="ExternalInput").ap(),
        "y": nc.dram_tensor("y", [S, D_MODEL], BF16,
                            kind="ExternalOutput").ap(),
        "warm": nc.dram_tensor("warm", [1, 512], F32,
                               kind="ExternalOutput").ap(),
        "wexp": nc.dram_tensor("wexp", [1, 512], F32,
                               kind="ExternalOutput").ap(),
    }
    with tile.TileContext(nc) as tc:
        with ExitStack() as ctx:
            _trace(ctx, tc, io)
    nc.compile()
    _CACHED_NC = nc
    return nc


def _core_inputs(q, k, v, W, b, core):
    bb, g = divmod(core, 4)
    hd0 = g * H_PER_CORE * D_K  # 256 per group
    ncol = H_PER_CORE * D_K
    bf = ml_dtypes.bfloat16

    qt = np.ascontiguousarray(q[bb, :, hd0:hd0 + ncol].T).reshape(2, 128, S)
    kt = np.ascontiguousarray(k[bb, :, hd0:hd0 + ncol].T).reshape(2, 128, S)
    v_sl = v[bb, :, hd0:hd0 + ncol].reshape(S, H_PER_CORE, D_K)
    va = np.concatenate(
        [np.ones((S, H_PER_CORE, 1), np.float32),
         np.zeros((S, H_PER_CORE, 63), np.float32), v_sl], axis=2
    ).reshape(JB, 128, H_PER_CORE * VA_W).transpose(1, 0, 2).reshape(
        128, JB * H_PER_CORE * VA_W)
    va = va.reshape(128, VA_CHUNK, JB // VA_CHUNK * H_PER_CORE * VA_W)
    va = np.ascontiguousarray(va.transpose(1, 0, 2))
    wt = np.ascontiguousarray(W[:, hd0:hd0 + ncol].T).reshape(2, 128, D_MODEL)
    return {
        "qt": qt.astype(bf),
        "kt": kt.astype(bf),
        "va": va.astype(bf),
        "wt": wt.astype(bf),
    }


def run(inputs, trace=False, trace_kwargs=None):
    from concourse.bass_utils import run_bass_kernel_spmd

    q = np.asarray(inputs["q"], np.float32)
    k = np.asarray(inputs["k"], np.float32)
    v = np.asarray(inputs["v"], np.float32)
    W = np.asarray(inputs["W"], np.float32)
    b = np.asarray(inputs["b"], np.float32)

    nc = _build()
    in_maps = [_core_inputs(q, k, v, W, b, c) for c in range(N_CORES)]
    res = run_bass_kernel_spmd(nc, in_maps, core_ids=list(range(N_CORES)),
                               trace=trace, **(trace_kwargs or {}))
    out = np.empty((B, S, D_MODEL), np.float32)
    for bb in range(B):
        acc = res.results[bb * 4 + 0]["y"].astype(np.float32)
        for g in range(1, 4):
            acc = acc + res.results[bb * 4 + g]["y"]
        out[bb] = acc + b[None, :]
    return out, res


def kernel(**inputs):
    out, _ = run(inputs)
    return out


# revision 16
# speedup vs baseline: 1.2660x; 1.0737x over previous
"""Multi-head attention (B=2, S=2048, D=1024, H=16) on 8 Trainium2 cores.

Sharding: core c handles batch b = c//4 and head group g = c%4 (4 heads).
Output projection is row-sharded over head dims; per-core partial outputs
are summed on the host (bias added on the host).

Design (v4, ACT-bound): the scalar engine's exp is the hard floor
(131072 cols/core @ 1.2GHz + ~352cyc/instr overhead ~= 147us), so the whole
schedule exists to keep ACT 100% busy on uniform [128,1024] exp tiles:

  - psS has THREE [128,1024] f32 slots (6 PSUM banks). With two QK tiles
    allocated per step, the 3-slot rotation opens each QK's WAR gate a full
    exp (~1.15us) before the pipeline needs it, so the two heads' K=64
    matmuls run as a concurrent row-group pair (full-array MAC activity)
    and ACT never waits on the in-order PE queue.
  - HAM (the PE clock gate) re-throttles unless the MAC array is nearly
    saturated per 3.4us window; garbage "fill" matmuls (K=128, written into
    the previous step's already-read S tile) top every step up to ~2.1us.
  - PV: prev-block head1 on steps 2..7, own head0 on steps 10..15 (start
    offsets clear the preceding norm chain's psO occupancy; V is augmented
    with a ones column so PSUM row 0 is the softmax denominator).
  - The whole output projection runs in the tail, deep-pipelined through
    the freed psS slots ([128,1024] Y tiles, copies alternating DVE/ACT).
norm: DVE reciprocal + gpsimd partition-broadcast + DVE multiply; head1
writes at_sb in place (rows 64-127), head0 stages + SBUF DMA (row shift).
"""

import sys

sys.path.insert(0, "/opt/trn_rl_repo")

from contextlib import ExitStack

import numpy as np
import ml_dtypes

import concourse.bass as bass
import concourse.tile as tile
from concourse import bacc, mybir

N_CORES = 8
B, S, D_MODEL = 2, 2048, 1024
NUM_HEADS, D_K = 16, 64
H_PER_CORE = 4
SCALE = D_K ** -0.5
IS = 1024                 # i-super width
JB = S // 128             # 16 j-blocks
VA_W = 128                # ones col 0, zeros 1-63, v at 64-127
VA_CHUNK = 4              # va split into 4 tiles of 4 j-blocks each
ET_BUFS = 44

F32 = mybir.dt.float32
BF16 = mybir.dt.bfloat16
AF = mybir.ActivationFunctionType
BLOCKS = [(0, 0), (1, 0), (0, 1024), (1, 1024)]  # (pair, i0), iw = 1024

MAC_TARGET = 2100         # ns of full-array MAC time to emit per step
FILL_MAC = 213

# per-step jbp batches: 16 j-blocks over steps 2..7 (head1 of prev block)
PV1_SCHED = {2: [0, 1, 2], 3: [3, 4, 5], 4: [6, 7, 8], 5: [9, 10, 11],
             6: [12, 13], 7: [14, 15]}
# ... and over steps 10..15 (own head0)
PV0_SCHED = {10: [0, 1, 2], 11: [3, 4, 5], 12: [6, 7, 8], 13: [9, 10, 11],
             14: [12, 13], 15: [14, 15]}


def ds(start, size):
    return slice(start, start + size)


def _trace(ctx: ExitStack, tc: tile.TileContext, io: dict):
    nc = tc.nc

    const = ctx.enter_context(tc.tile_pool(name="const", bufs=1))
    etp = ctx.enter_context(tc.tile_pool(name="et", bufs=ET_BUFS))
    normp = ctx.enter_context(tc.tile_pool(name="norm", bufs=2))
    atp = ctx.enter_context(tc.tile_pool(name="at", bufs=1))
    youtp = ctx.enter_context(tc.tile_pool(name="yout", bufs=6))
    miscp = ctx.enter_context(tc.tile_pool(name="misc", bufs=2))
    psS = ctx.enter_context(tc.tile_pool(name="psS", bufs=3, space="PSUM"))
    psO = ctx.enter_context(tc.tile_pool(name="psO", bufs=1, space="PSUM"))

    # ---- resident inputs (order matters: earliest-needed first) ----
    kt_sb = [const.tile([128, S], BF16, tag=f"kt{p}", name=f"kt{p}")
             for p in range(2)]
    qt_sb = [const.tile([128, S], BF16, tag=f"qt{p}", name=f"qt{p}")
             for p in range(2)]
    va_sb = [const.tile([128, JB // VA_CHUNK * H_PER_CORE * VA_W], BF16,
                        tag=f"va{c}", name=f"va{c}") for c in range(VA_CHUNK)]
    nc.sync.dma_start(kt_sb[0][:, 0:512], io["kt"][0][:, 0:512])
    nc.sync.dma_start(qt_sb[0][:, 0:IS], io["qt"][0][:, 0:IS])
    nc.sync.dma_start(va_sb[0][:], io["va"][0])
    nc.sync.dma_start(kt_sb[0][:, 512:S], io["kt"][0][:, 512:S])
    nc.sync.dma_start(va_sb[1][:], io["va"][1])
    nc.sync.dma_start(kt_sb[1][:], io["kt"][1])
    nc.sync.dma_start(qt_sb[1][:, 0:IS], io["qt"][1][:, 0:IS])
    nc.sync.dma_start(va_sb[2][:], io["va"][2])
    nc.sync.dma_start(va_sb[3][:], io["va"][3])
    wt_sb = []
    for p in range(2):
        t = const.tile([128, D_MODEL], BF16, tag=f"wt{p}")
        nc.sync.dma_start(t[:], io["wt"][p])
        wt_sb.append(t)
    nc.sync.dma_start(qt_sb[0][:, IS:S], io["qt"][0][:, IS:S])
    nc.sync.dma_start(qt_sb[1][:, IS:S], io["qt"][1][:, IS:S])
    at_sb = [atp.tile([128, S], BF16, tag=f"at{p}", name=f"at{p}")
             for p in range(2)]

    # ---- warmup ----
    # exp-table preload on ACT (reads the first kt sliver, so the ~2.7us
    # table load overlaps input DMA), then 4 full-array matmuls for HAM
    wexp = miscp.tile([1, 512], F32, tag="warm_exp", name="warm_exp")
    nc.scalar.activation(wexp[:], kt_sb[0][0:1, 0:512], AF.Exp, scale=SCALE)
    nc.sync.dma_start(io["wexp"][:], wexp[:])
    wps = psS.tile([128, IS], F32, tag="S", name="warm_ps")
    for _ in range(4):
        nc.tensor.matmul(wps[:, 0:512], kt_sb[0][:, 0:128],
                         kt_sb[0][:, 0:512],
                         start=True, stop=True, skip_group_check=True)
    wsb = miscp.tile([1, 512], F32, tag="warm_out", name="warm_out")
    nc.vector.tensor_copy(wsb[:], wps[0:1, 0:512])
    nc.sync.dma_start(io["warm"][:], wsb[:])

    ET = {}    # (block_idx, h2) -> list of 16 E tiles
    PSO = {}   # (block_idx, h2) -> psum tile
    dead_sp = [wps]  # S tiles whose exp already ran: fill targets

    def emit_qk_exp(t, jb):
        pr, i0 = BLOCKS[t]
        # interleave the two heads' matmuls: distinct PE row-groups run
        # concurrently (full-array MAC activity keeps HAM at 8/8)
        sps = [psS.tile([128, IS], F32, tag="S", name="sp") for _ in range(2)]
        for nch in range(2):
            for h2 in range(2):
                nc.tensor.matmul(
                    sps[h2][:, ds(nch * 512, 512)],
                    kt_sb[pr][ds(h2 * 64, 64), ds(jb * 128, 128)],
                    qt_sb[pr][ds(h2 * 64, 64), ds(i0 + nch * 512, 512)],
                    start=True, stop=True,
                )
        for h2 in range(2):
            e = etp.tile([128, IS], BF16, tag="et", name="e")
            nc.scalar.activation(e[:], sps[h2][:], AF.Exp, scale=SCALE)
            ET[(t, h2)][jb] = e
            dead_sp.append(sps[h2])

    def emit_pv(t, h2, jbps):
        pr, i0 = BLOCKS[t]
        h = pr * 2 + h2
        if (t, h2) not in PSO:
            PSO[(t, h2)] = psO.tile([128, IS], F32, tag="O", name="psO")
        O = PSO[(t, h2)]
        for jbp in jbps:
            va = va_sb[jbp // VA_CHUNK]
            vo = (jbp % VA_CHUNK) * H_PER_CORE * VA_W + h * VA_W
            for nch in range(2):
                nc.tensor.matmul(
                    O[0:128, ds(nch * 512, 512)],
                    va[:, ds(vo, VA_W)],
                    ET[(t, h2)][jbp][:, ds(nch * 512, 512)],
                    start=(jbp == 0), stop=(jbp == JB - 1),
                    skip_group_check=True,
                )

    def emit_norm(t, h2, split=False):
        pr, i0 = BLOCKS[t]
        O = PSO[(t, h2)]
        chunks = [(0, 512), (512, 512)] if split else [(0, IS)]
        for off, w in chunks:
            rr = normp.tile([1, w], F32, tag="rr", name="rr")
            nc.vector.reciprocal_approx_fast(rr[:], O[0:1, ds(off, w)])
            bc = normp.tile([128, w], F32, tag="bc", name="bc")
            nc.gpsimd.partition_broadcast(bc[:], rr[0:1, :])
            if h2 == 1:
                # head1's dims are rows 64-127 of at_sb: write in place
                nc.vector.tensor_mul(
                    at_sb[pr][ds(64, 64), ds(i0 + off, w)],
                    O[64:128, ds(off, w)], bc[64:128, :])
            else:
                nm = normp.tile([128, w], BF16, tag="nm", name="nm")
                nc.vector.tensor_mul(nm[64:128, :], O[64:128, ds(off, w)],
                                     bc[64:128, :])
                nc.sync.dma_start(
                    at_sb[pr][ds(0, 64), ds(i0 + off, w)],
                    nm[64:128, :])
        del ET[(t, h2)]

    def emit_proj(ic, eng="vector"):
        # one i-chunk, full 1024 output columns, through a free psS slot
        Y = psS.tile([128, IS], F32, tag="S", name="Ypj")
        for moch in range(2):
            for hd2 in range(2):
                nc.tensor.matmul(
                    Y[:, ds(moch * 512, 512)],
                    at_sb[hd2][:, ds(ic * 128, 128)],
                    wt_sb[hd2][:, ds(moch * 512, 512)],
                    start=(hd2 == 0), stop=(hd2 == 1),
                    skip_group_check=True,
                )
        ysb = youtp.tile([128, IS], BF16, tag="y")
        if eng == "vector":
            nc.vector.tensor_copy(ysb[:], Y[:])
        else:
            nc.scalar.copy(ysb[:], Y[:])
        nc.sync.dma_start(io["y"][ds(ic * 128, 128), :], ysb[:])

    def emit_fill(mac_ns):
        # full-array garbage matmuls (K=128, 512 cols): keep the PE's MAC
        # duty above the HAM re-throttle threshold. Target the previous
        # step's h1 S-tile: its exp is (just) done and its psS slot is not
        # re-allocated until the next step's h0 QK, so the pool's slot-reuse
        # dependency keeps everything ordered.
        tgt = dead_sp[-3] if len(dead_sp) >= 3 else dead_sp[0]
        n = max(0, round(mac_ns / FILL_MAC))
        for i in range(n):
            nc.tensor.matmul(tgt[:, ds(512 * (i % 2), 512)],
                             kt_sb[0][:, 0:128], kt_sb[0][:, 0:512],
                             start=True, stop=True, skip_group_check=True)

    for t in range(len(BLOCKS)):
        for h2 in range(2):
            ET[(t, h2)] = [None] * JB
        for jb in range(JB):
            emit_qk_exp(t, jb)
            mac = 426
            if t == 0:
                # no previous-block PV: spread own head0 one j-block per step
                if 1 <= jb < JB - 1:
                    emit_pv(0, 0, [jb - 1])
                    mac += 426
                elif jb == JB - 1:
                    emit_pv(0, 0, [14, 15])
                    emit_norm(0, 0)
                    mac += 852
            else:
                if jb in PV1_SCHED:
                    jbps = PV1_SCHED[jb]
                    emit_pv(t - 1, 1, jbps)
                    mac += 426 * len(jbps)
                    if jb == 7:
                        emit_norm(t - 1, 1)
                if jb in PV0_SCHED:
                    jbps = PV0_SCHED[jb]
                    emit_pv(t, 0, jbps)
                    mac += 426 * len(jbps)
                    if jb == JB - 1:
                        emit_norm(t, 0)
            emit_fill(MAC_TARGET - mac)

    # tail: last block's head-1 PV (psO frees only after norm(3,0)'s chain,
    # so fills bridge the wait), split norm, then the entire projection
    # pipelined through the freed psS slots
    emit_fill(4200)
    emit_pv(3, 1, list(range(JB)))
    emit_norm(3, 1, split=True)
    for ic in range(16):
        emit_proj(ic, eng=("scalar" if ic % 2 else "vector"))


_CACHED_NC = None


def _build():
    global _CACHED_NC
    if _CACHED_NC is not None:
        return _CACHED_NC
    nc = bacc.Bacc("TRN2", target_bir_lowering=False, debug=False,
                   num_devices=N_CORES)
    va_cols = JB // VA_CHUNK * H_PER_CORE * VA_W
    io = {
        "qt": nc.dram_tensor("qt", [2, 128, S], BF16,
                             kind="ExternalInput").ap(),
        "kt": nc.dram_tensor("kt", [2, 128, S], BF16,
                             kind="ExternalInput").ap(),
        "va": nc.dram_tensor("va", [VA_CHUNK, 128, va_cols], BF16,
                             kind="ExternalInput").ap(),
        "wt": nc.dram_tensor("wt", [2, 128, D_MODEL], BF16,
                             kind="ExternalInput").ap(),
        "y": nc.dram_tensor("y", [S, D_MODEL], BF16,
                            kind="ExternalOutput").ap(),
        "warm": nc.dram_tensor("warm", [1, 512], F32,
                               kind="ExternalOutput").ap(),
        "wexp": nc.dram_tensor("wexp", [1, 512], F32,
                               kind="ExternalOutput").ap(),
    }
    with tile.TileContext(nc) as tc:
        with ExitStack() as ctx:
            _trace(ctx, tc, io)
    nc.compile()
    _CACHED_NC = nc
    return nc


def _core_inputs(q, k, v, W, b, core):
    bb, g = divmod(core, 4)
    hd0 = g * H_PER_CORE * D_K  # 256 per group
    ncol = H_PER_CORE * D_K
    bf = ml_dtypes.bfloat16

    qt = np.ascontiguousarray(q[bb, :, hd0:hd0 + ncol].T).reshape(2, 128, S)
    kt = np.ascontiguousarray(k[bb, :, hd0:hd0 + ncol].T).reshape(2, 128, S)
    v_sl = v[bb, :, hd0:hd0 + ncol].reshape(S, H_PER_CORE, D_K)
    va = np.concatenate(
        [np.ones((S, H_PER_CORE, 1), np.float32),
         np.zeros((S, H_PER_CORE, 63), np.float32), v_sl], axis=2
    ).reshape(JB, 128, H_PER_CORE * VA_W).transpose(1, 0, 2).reshape(
        128, JB * H_PER_CORE * VA_W)
    va = va.reshape(128, VA_CHUNK, JB // VA_CHUNK * H_PER_CORE * VA_W)
    va = np.ascontiguousarray(va.transpose(1, 0, 2))
    wt = np.ascontiguousarray(W[:, hd0:hd0 + ncol].T).reshape(2, 128, D_MODEL)
    return {
        "qt": qt.astype(bf),
        "kt": kt.astype(bf),
        "va": va.astype(bf),
        "wt": wt.astype(bf),
    }


def run(inputs, trace=False, trace_kwargs=None):
    from concourse.bass_utils import run_bass_kernel_spmd

    q = np.asarray(inputs["q"], np.float32)
    k = np.asarray(inputs["k"], np.float32)
    v = np.asarray(inputs["v"], np.float32)
    W = np.asarray(inputs["W"], np.float32)
    b = np.asarray(inputs["b"], np.float32)

    nc = _build()
    in_maps = [_core_inputs(q, k, v, W, b, c) for c in range(N_CORES)]
    res = run_bass_kernel_spmd(nc, in_maps, core_ids=list(range(N_CORES)),
                               trace=trace, **(trace_kwargs or {}))
    out = np.empty((B, S, D_MODEL), np.float32)
    for bb in range(B):
        acc = res.results[bb * 4 + 0]["y"].astype(np.float32)
        for g in range(1, 4):
            acc = acc + res.results[bb * 4 + g]["y"].astype(np.float32)
        out[bb] = acc + b[None, :]
    return out, res


def kernel(**inputs):
    out, _ = run(inputs)
    return out


# revision 19
# speedup vs baseline: 1.3356x; 1.0550x over previous
"""Multi-head attention (B=2, S=2048, D=1024, H=16) on 8 Trainium2 cores.

Sharding: core c handles batch b = c//4 and head group g = c%4 (4 heads).
Output projection is row-sharded over head dims; per-core partial outputs
are summed on the host (bias added on the host).

Design (v4, ACT-bound): the scalar engine's exp is the hard floor
(131072 cols/core @ 1.2GHz + ~352cyc/instr overhead ~= 147us), so the whole
schedule exists to keep ACT 100% busy on uniform [128,1024] exp tiles:

  - psS has THREE [128,1024] f32 slots (6 PSUM banks). With two QK tiles
    allocated per step, the 3-slot rotation opens each QK's WAR gate a full
    exp (~1.15us) before the pipeline needs it, so the two heads' K=64
    matmuls run as a concurrent row-group pair (full-array MAC activity)
    and ACT never waits on the in-order PE queue.
  - HAM (the PE clock gate) re-throttles unless the MAC array is nearly
    saturated per 3.4us window; garbage "fill" matmuls (K=128, written into
    the previous step's already-read S tile) top every step up to ~2.1us.
  - PV: prev-block head1 on steps 2..7, own head0 on steps 10..15 (start
    offsets clear the preceding norm chain's psO occupancy; V is augmented
    with a ones column so PSUM row 0 is the softmax denominator).
  - The whole output projection runs in the tail, deep-pipelined through
    the freed psS slots ([128,1024] Y tiles, copies alternating DVE/ACT).
norm: DVE reciprocal + gpsimd partition-broadcast + DVE multiply; head1
writes at_sb in place (rows 64-127), head0 stages + SBUF DMA (row shift).
"""

import sys

sys.path.insert(0, "/opt/trn_rl_repo")

from contextlib import ExitStack

import numpy as np
import ml_dtypes

import concourse.bass as bass
import concourse.tile as tile
from concourse import bacc, mybir

N_CORES = 8
B, S, D_MODEL = 2, 2048, 1024
NUM_HEADS, D_K = 16, 64
H_PER_CORE = 4
SCALE = D_K ** -0.5
IS = 1024                 # i-super width
JB = S // 128             # 16 j-blocks
VA_W = 128                # ones col 0, zeros 1-63, v at 64-127
VA_CHUNK = 4              # va split into 4 tiles of 4 j-blocks each
ET_BUFS = 44

F32 = mybir.dt.float32
BF16 = mybir.dt.bfloat16
AF = mybir.ActivationFunctionType
BLOCKS = [(0, 0), (1, 0), (0, 1024), (1, 1024)]  # (pair, i0), iw = 1024

PE_TARGET = 2220          # ns of PE time to emit per step (ACT step ~2292)
QK_PE = 713               # measured QK quad wall (LDW stagger included)
PV_PE = 426               # per j-block (2 chunk matmuls)
FILL_MAC = 213

# per-step jbp batches: prev-block head1 over steps 3..8 (start clears the
# preceding norm chain's psO occupancy), own head0 over steps 10..15; the
# last j-block + norm run at the NEXT block's step 1, after its gating exp
# has certainly retired, so the in-order PE queue never stalls on them
PV1_SCHED = {3: [0, 1, 2], 4: [3, 4, 5], 5: [6, 7, 8], 6: [9, 10, 11],
             7: [12, 13], 8: [14, 15]}
PV0_SCHED = {10: [0, 1, 2], 11: [3, 4, 5], 12: [6, 7, 8], 13: [9, 10, 11],
             14: [12, 13], 15: [14]}


def ds(start, size):
    return slice(start, start + size)


def _trace(ctx: ExitStack, tc: tile.TileContext, io: dict):
    nc = tc.nc

    const = ctx.enter_context(tc.tile_pool(name="const", bufs=1))
    etp = ctx.enter_context(tc.tile_pool(name="et", bufs=ET_BUFS))
    normp = ctx.enter_context(tc.tile_pool(name="norm", bufs=2))
    atp = ctx.enter_context(tc.tile_pool(name="at", bufs=1))
    youtp = ctx.enter_context(tc.tile_pool(name="yout", bufs=6))
    miscp = ctx.enter_context(tc.tile_pool(name="misc", bufs=2))
    psS = ctx.enter_context(tc.tile_pool(name="psS", bufs=3, space="PSUM"))
    psO = ctx.enter_context(tc.tile_pool(name="psO", bufs=1, space="PSUM"))

    # ---- resident inputs (order matters: earliest-needed first) ----
    kt_sb = [const.tile([128, S], BF16, tag=f"kt{p}", name=f"kt{p}")
             for p in range(2)]
    qt_sb = [const.tile([128, S], BF16, tag=f"qt{p}", name=f"qt{p}")
             for p in range(2)]
    va_sb = [const.tile([128, JB // VA_CHUNK * H_PER_CORE * VA_W], BF16,
                        tag=f"va{c}", name=f"va{c}") for c in range(VA_CHUNK)]
    nc.sync.dma_start(kt_sb[0][:, 0:512], io["kt"][0][:, 0:512])
    nc.sync.dma_start(qt_sb[0][:, 0:IS], io["qt"][0][:, 0:IS])
    nc.sync.dma_start(va_sb[0][:], io["va"][0])
    nc.sync.dma_start(kt_sb[0][:, 512:S], io["kt"][0][:, 512:S])
    nc.sync.dma_start(va_sb[1][:], io["va"][1])
    nc.sync.dma_start(kt_sb[1][:], io["kt"][1])
    nc.sync.dma_start(qt_sb[1][:, 0:IS], io["qt"][1][:, 0:IS])
    nc.sync.dma_start(va_sb[2][:], io["va"][2])
    nc.sync.dma_start(va_sb[3][:], io["va"][3])
    wt_sb = []
    for p in range(2):
        t = const.tile([128, D_MODEL], BF16, tag=f"wt{p}")
        nc.sync.dma_start(t[:], io["wt"][p])
        wt_sb.append(t)
    nc.sync.dma_start(qt_sb[0][:, IS:S], io["qt"][0][:, IS:S])
    nc.sync.dma_start(qt_sb[1][:, IS:S], io["qt"][1][:, IS:S])
    at_sb = [atp.tile([128, S], BF16, tag=f"at{p}", name=f"at{p}")
             for p in range(2)]

    # ---- warmup ----
    # exp-table preload on ACT (reads the first kt sliver, so the ~2.7us
    # table load overlaps input DMA), then 4 full-array matmuls for HAM
    wexp = miscp.tile([1, 512], F32, tag="warm_exp", name="warm_exp")
    nc.scalar.activation(wexp[:], kt_sb[0][0:1, 0:512], AF.Exp, scale=SCALE)
    nc.sync.dma_start(io["wexp"][:], wexp[:])
    wps = psS.tile([128, IS], F32, tag="S", name="warm_ps")
    for _ in range(4):
        nc.tensor.matmul(wps[:, 0:512], kt_sb[0][:, 0:128],
                         kt_sb[0][:, 0:512],
                         start=True, stop=True, skip_group_check=True)
    wsb = miscp.tile([1, 512], F32, tag="warm_out", name="warm_out")
    nc.vector.tensor_copy(wsb[:], wps[0:1, 0:512])
    nc.sync.dma_start(io["warm"][:], wsb[:])

    ET = {}    # (block_idx, h2) -> list of 16 E tiles
    PSO = {}   # (block_idx, h2) -> psum tile
    dead_sp = [wps]  # S tiles whose exp already ran: fill targets

    def emit_qk_exp(t, jb):
        pr, i0 = BLOCKS[t]
        # interleave the two heads' matmuls: distinct PE row-groups run
        # concurrently (full-array MAC activity keeps HAM at 8/8)
        sps = [psS.tile([128, IS], F32, tag="S", name="sp") for _ in range(2)]
        for nch in range(2):
            for h2 in range(2):
                nc.tensor.matmul(
                    sps[h2][:, ds(nch * 512, 512)],
                    kt_sb[pr][ds(h2 * 64, 64), ds(jb * 128, 128)],
                    qt_sb[pr][ds(h2 * 64, 64), ds(i0 + nch * 512, 512)],
                    start=True, stop=True,
                )
        for h2 in range(2):
            e = etp.tile([128, IS], BF16, tag="et", name="e")
            nc.scalar.activation(e[:], sps[h2][:], AF.Exp, scale=SCALE)
            ET[(t, h2)][jb] = e
            dead_sp.append(sps[h2])

    def emit_pv(t, h2, jbps, pool=None):
        pr, i0 = BLOCKS[t]
        h = pr * 2 + h2
        if (t, h2) not in PSO:
            pool = pool or psO
            tag = "O" if pool is psO else "S"
            PSO[(t, h2)] = pool.tile([128, IS], F32, tag=tag, name="psO")
        O = PSO[(t, h2)]
        for jbp in jbps:
            va = va_sb[jbp // VA_CHUNK]
            vo = (jbp % VA_CHUNK) * H_PER_CORE * VA_W + h * VA_W
            for nch in range(2):
                nc.tensor.matmul(
                    O[0:128, ds(nch * 512, 512)],
                    va[:, ds(vo, VA_W)],
                    ET[(t, h2)][jbp][:, ds(nch * 512, 512)],
                    start=(jbp == 0), stop=(jbp == JB - 1),
                    skip_group_check=True,
                )

    def emit_norm(t, h2, split=False):
        pr, i0 = BLOCKS[t]
        O = PSO[(t, h2)]
        chunks = [(0, 512), (512, 512)] if split else [(0, IS)]
        for off, w in chunks:
            rr = normp.tile([1, w], F32, tag="rr", name="rr")
            nc.vector.reciprocal_approx_fast(rr[:], O[0:1, ds(off, w)])
            bc = normp.tile([128, w], F32, tag="bc", name="bc")
            nc.gpsimd.partition_broadcast(bc[:], rr[0:1, :])
            if h2 == 1:
                # head1's dims are rows 64-127 of at_sb: write in place
                nc.vector.tensor_mul(
                    at_sb[pr][ds(64, 64), ds(i0 + off, w)],
                    O[64:128, ds(off, w)], bc[64:128, :])
            else:
                nm = normp.tile([128, w], BF16, tag="nm", name="nm")
                nc.vector.tensor_mul(nm[64:128, :], O[64:128, ds(off, w)],
                                     bc[64:128, :])
                nc.sync.dma_start(
                    at_sb[pr][ds(0, 64), ds(i0 + off, w)],
                    nm[64:128, :])
        del ET[(t, h2)]

    def emit_proj(ic, eng="vector"):
        # one i-chunk, full 1024 output columns, through a free psS slot
        Y = psS.tile([128, IS], F32, tag="S", name="Ypj")
        for moch in range(2):
            for hd2 in range(2):
                nc.tensor.matmul(
                    Y[:, ds(moch * 512, 512)],
                    at_sb[hd2][:, ds(ic * 128, 128)],
                    wt_sb[hd2][:, ds(moch * 512, 512)],
                    start=(hd2 == 0), stop=(hd2 == 1),
                    skip_group_check=True,
                )
        ysb = youtp.tile([128, IS], BF16, tag="y")
        if eng == "vector":
            nc.vector.tensor_copy(ysb[:], Y[:])
        else:
            nc.scalar.copy(ysb[:], Y[:])
        nc.sync.dma_start(io["y"][ds(ic * 128, 128), :], ysb[:])

    def emit_fill(mac_ns):
        # full-array garbage matmuls (K=128, 512 cols): keep the PE's MAC
        # duty above the HAM re-throttle threshold. Target the previous
        # step's h1 S-tile: its exp is (just) done and its psS slot is not
        # re-allocated until the next step's h0 QK, so the pool's slot-reuse
        # dependency keeps everything ordered.
        tgt = dead_sp[-3] if len(dead_sp) >= 3 else dead_sp[0]
        n = max(0, round(mac_ns / FILL_MAC))
        for i in range(n):
            nc.tensor.matmul(tgt[:, ds(512 * (i % 2), 512)],
                             kt_sb[0][:, 0:128], kt_sb[0][:, 0:512],
                             start=True, stop=True, skip_group_check=True)

    for t in range(len(BLOCKS)):
        for h2 in range(2):
            ET[(t, h2)] = [None] * JB
        for jb in range(JB):
            emit_qk_exp(t, jb)
            pe = QK_PE
            if jb == 1 and t >= 1:
                # previous block's deferred last j-block + its norm
                emit_pv(t - 1, 0, [15])
                emit_norm(t - 1, 0)
                pe += PV_PE
            if t == 0:
                # no previous-block PV: spread own head0 one j-block per step
                if 1 <= jb < JB - 1:
                    emit_pv(0, 0, [jb - 1])
                    pe += PV_PE
                elif jb == JB - 1:
                    emit_pv(0, 0, [14])
                    pe += PV_PE
            else:
                if jb in PV1_SCHED:
                    jbps = PV1_SCHED[jb]
                    emit_pv(t - 1, 1, jbps)
                    pe += PV_PE * len(jbps)
                    if jb == 8:
                        emit_norm(t - 1, 1)
                if jb in PV0_SCHED:
                    jbps = PV0_SCHED[jb]
                    emit_pv(t, 0, jbps)
                    pe += PV_PE * len(jbps)
            emit_fill(PE_TARGET - pe)

    # tail: block 3's deferred head0 j-block + norm, then its head-1 PV
    # accumulating in a freed psS slot (no psO-chain wait), split norm, then
    # the entire projection pipelined through the remaining psS slots
    emit_fill(1500)  # bridge until exp(3,15,*) retire
    emit_pv(3, 0, [15])
    emit_norm(3, 0)
    emit_pv(3, 1, list(range(JB)), pool=psS)
    emit_norm(3, 1, split=True)
    for ic in range(16):
        emit_proj(ic, eng=("scalar" if ic % 2 else "vector"))


_CACHED_NC = None


def _build():
    global _CACHED_NC
    if _CACHED_NC is not None:
        return _CACHED_NC
    nc = bacc.Bacc("TRN2", target_bir_lowering=False, debug=False,
                   num_devices=N_CORES)
    va_cols = JB // VA_CHUNK * H_PER_CORE * VA_W
    io = {
        "qt": nc.dram_tensor("qt", [2, 128, S], BF16,
                             kind="ExternalInput").ap(),
        "kt": nc.dram_tensor("kt", [2, 128, S], BF16,
                             kind="ExternalInput").ap(),
        "va": nc.dram_tensor("va", [VA_CHUNK, 128, va_cols], BF16,
                             kind="ExternalInput").ap(),
        "wt": nc.dram_tensor("wt", [2, 128, D_MODEL], BF16,
                             kind="ExternalInput").ap(),
        "y": nc.dram_tensor("y", [S, D_MODEL], BF16,
                            kind="ExternalOutput").ap(),
        "warm": nc.dram_tensor("warm", [1, 512], F32,
                               kind="ExternalOutput").ap(),
        "wexp": nc.dram_tensor("wexp", [1, 512], F32,
                               kind="ExternalOutput").ap(),
    }
    with tile.TileContext(nc) as tc:
        with ExitStack() as ctx:
            _trace(ctx, tc, io)
    nc.compile()
    _CACHED_NC = nc
    return nc


def _core_inputs(q, k, v, W, b, core):
    bb, g = divmod(core, 4)
    hd0 = g * H_PER_CORE * D_K  # 256 per group
    ncol = H_PER_CORE * D_K
    bf = ml_dtypes.bfloat16

    qt = np.ascontiguousarray(q[bb, :, hd0:hd0 + ncol].T).reshape(2, 128, S)
    kt = np.ascontiguousarray(k[bb, :, hd0:hd0 + ncol].T).reshape(2, 128, S)
    v_sl = v[bb, :, hd0:hd0 + ncol].reshape(S, H_PER_CORE, D_K)
    va = np.concatenate(
        [np.ones((S, H_PER_CORE, 1), np.float32),
         np.zeros((S, H_PER_CORE, 63), np.float32), v_sl], axis=2
    ).reshape(JB, 128, H_PER_CORE * VA_W).transpose(1, 0, 2).reshape(
        128, JB * H_PER_CORE * VA_W)
    va = va.reshape(128, VA_CHUNK, JB // VA_CHUNK * H_PER_CORE * VA_W)
    va = np.ascontiguousarray(va.transpose(1, 0, 2))
    wt = np.ascontiguousarray(W[:, hd0:hd0 + ncol].T).reshape(2, 128, D_MODEL)
    return {
        "qt": qt.astype(bf),
        "kt": kt.astype(bf),
        "va": va.astype(bf),
        "wt": wt.astype(bf),
    }


def run(inputs, trace=False, trace_kwargs=None):
    from concourse.bass_utils import run_bass_kernel_spmd

    q = np.asarray(inputs["q"], np.float32)
    k = np.asarray(inputs["k"], np.float32)
    v = np.asarray(inputs["v"], np.float32)
    W = np.asarray(inputs["W"], np.float32)
    b = np.asarray(inputs["b"], np.float32)

    nc = _build()
    in_maps = [_core_inputs(q, k, v, W, b, c) for c in range(N_CORES)]
    res = run_bass_kernel_spmd(nc, in_maps, core_ids=list(range(N_CORES)),
                               trace=trace, **(trace_kwargs or {}))
    out = np.empty((B, S, D_MODEL), np.float32)
    for bb in range(B):
        acc = res.results[bb * 4 + 0]["y"].astype(np.float32)
        for g in range(1, 4):
            acc = acc + res.results[bb * 4 + g]["y"].astype(np.float32)
        out[bb] = acc + b[None, :]
    return out, res


def kernel(**inputs):
    out, _ = run(inputs)
    return out


# revision 22
# speedup vs baseline: 1.3410x; 1.0040x over previous
"""Multi-head attention (B=2, S=2048, D=1024, H=16) on 8 Trainium2 cores.

Sharding: core c handles batch b = c//4 and head group g = c%4 (4 heads).
Output projection is row-sharded over head dims; per-core partial outputs
are summed on the host (bias added on the host).

Design (v4, ACT-bound): the scalar engine's exp is the hard floor
(131072 cols/core @ 1.2GHz + ~352cyc/instr overhead ~= 147us), so the whole
schedule exists to keep ACT 100% busy on uniform [128,1024] exp tiles:

  - psS has THREE [128,1024] f32 slots (6 PSUM banks). With two QK tiles
    allocated per step, the 3-slot rotation opens each QK's WAR gate a full
    exp (~1.15us) before the pipeline needs it, so the two heads' K=64
    matmuls run as a concurrent row-group pair (full-array MAC activity)
    and ACT never waits on the in-order PE queue.
  - HAM (the PE clock gate) re-throttles unless the MAC array is nearly
    saturated per 3.4us window; garbage "fill" matmuls (K=128, written into
    the previous step's already-read S tile) top every step up to ~2.1us.
  - PV: prev-block head1 on steps 2..7, own head0 on steps 10..15 (start
    offsets clear the preceding norm chain's psO occupancy; V is augmented
    with a ones column so PSUM row 0 is the softmax denominator).
  - The whole output projection runs in the tail, deep-pipelined through
    the freed psS slots ([128,1024] Y tiles, copies alternating DVE/ACT).
norm: DVE reciprocal + gpsimd partition-broadcast + DVE multiply; head1
writes at_sb in place (rows 64-127), head0 stages + SBUF DMA (row shift).
"""

import sys

sys.path.insert(0, "/opt/trn_rl_repo")

from contextlib import ExitStack

import numpy as np
import ml_dtypes

import concourse.bass as bass
import concourse.tile as tile
from concourse import bacc, mybir

N_CORES = 8
B, S, D_MODEL = 2, 2048, 1024
NUM_HEADS, D_K = 16, 64
H_PER_CORE = 4
SCALE = D_K ** -0.5
IS = 1024                 # i-super width
JB = S // 128             # 16 j-blocks
VA_W = 128                # ones col 0, zeros 1-63, v at 64-127
VA_CHUNK = 4              # va split into 4 tiles of 4 j-blocks each
ET_BUFS = 44

F32 = mybir.dt.float32
BF16 = mybir.dt.bfloat16
AF = mybir.ActivationFunctionType
BLOCKS = [(0, 0), (1, 0), (0, 1024), (1, 1024)]  # (pair, i0), iw = 1024

PE_TARGET = 2160          # ns of PE time to emit per step (ACT step ~2250)
QK_PE = 713               # measured QK quad wall (LDW stagger included)
PV_PE = 426               # per j-block (2 chunk matmuls)
FILL_MAC = 213

# per-step jbp batches: prev-block head1 over steps 3..8 (start clears the
# preceding norm chain's psO occupancy), own head0 over steps 10..15; the
# last j-block + norm run at the NEXT block's step 1, after its gating exp
# has certainly retired, so the in-order PE queue never stalls on them
PV1_SCHED = {3: [0, 1, 2], 4: [3, 4, 5], 5: [6, 7, 8], 6: [9, 10, 11],
             7: [12, 13], 8: [14, 15]}
PV0_SCHED = {10: [0, 1, 2], 11: [3, 4, 5], 12: [6, 7, 8], 13: [9, 10, 11],
             14: [12, 13], 15: [14]}


def ds(start, size):
    return slice(start, start + size)


def _trace(ctx: ExitStack, tc: tile.TileContext, io: dict):
    nc = tc.nc

    const = ctx.enter_context(tc.tile_pool(name="const", bufs=1))
    etp = ctx.enter_context(tc.tile_pool(name="et", bufs=ET_BUFS))
    normp = ctx.enter_context(tc.tile_pool(name="norm", bufs=2))
    atp = ctx.enter_context(tc.tile_pool(name="at", bufs=1))
    youtp = ctx.enter_context(tc.tile_pool(name="yout", bufs=6))
    miscp = ctx.enter_context(tc.tile_pool(name="misc", bufs=2))
    psS = ctx.enter_context(tc.tile_pool(name="psS", bufs=3, space="PSUM"))
    psO = ctx.enter_context(tc.tile_pool(name="psO", bufs=1, space="PSUM"))

    # ---- resident inputs (order matters: earliest-needed first) ----
    kt_sb = [const.tile([128, S], BF16, tag=f"kt{p}", name=f"kt{p}")
             for p in range(2)]
    qt_sb = [const.tile([128, S], BF16, tag=f"qt{p}", name=f"qt{p}")
             for p in range(2)]
    va_sb = [const.tile([128, JB // VA_CHUNK * H_PER_CORE * VA_W], BF16,
                        tag=f"va{c}", name=f"va{c}") for c in range(VA_CHUNK)]
    nc.sync.dma_start(kt_sb[0][:, 0:512], io["kt"][0][:, 0:512])
    nc.sync.dma_start(qt_sb[0][:, 0:IS], io["qt"][0][:, 0:IS])
    nc.sync.dma_start(va_sb[0][:], io["va"][0])
    nc.sync.dma_start(kt_sb[0][:, 512:S], io["kt"][0][:, 512:S])
    nc.sync.dma_start(va_sb[1][:], io["va"][1])
    nc.sync.dma_start(kt_sb[1][:], io["kt"][1])
    nc.sync.dma_start(qt_sb[1][:, 0:IS], io["qt"][1][:, 0:IS])
    nc.sync.dma_start(va_sb[2][:], io["va"][2])
    nc.sync.dma_start(va_sb[3][:], io["va"][3])
    wt_sb = []
    for p in range(2):
        t = const.tile([128, D_MODEL], BF16, tag=f"wt{p}")
        nc.sync.dma_start(t[:], io["wt"][p])
        wt_sb.append(t)
    nc.sync.dma_start(qt_sb[0][:, IS:S], io["qt"][0][:, IS:S])
    nc.sync.dma_start(qt_sb[1][:, IS:S], io["qt"][1][:, IS:S])
    at_sb = [atp.tile([128, S], BF16, tag=f"at{p}", name=f"at{p}")
             for p in range(2)]

    # ---- warmup ----
    # exp-table preload on ACT (reads the first kt sliver, so the ~2.7us
    # table load overlaps input DMA), then 4 full-array matmuls for HAM
    wexp = miscp.tile([1, 512], F32, tag="warm_exp", name="warm_exp")
    nc.scalar.activation(wexp[:], kt_sb[0][0:1, 0:512], AF.Exp, scale=SCALE)
    nc.sync.dma_start(io["wexp"][:], wexp[:])
    wps = psS.tile([128, IS], F32, tag="S", name="warm_ps")
    nc.tensor.matmul(wps[:, 0:512], kt_sb[0][:, 0:128], kt_sb[0][:, 0:512],
                     start=True, stop=True, skip_group_check=True)
    wsb = miscp.tile([1, 512], F32, tag="warm_out", name="warm_out")
    nc.vector.tensor_copy(wsb[:], wps[0:1, 0:512])
    nc.sync.dma_start(io["warm"][:], wsb[:])

    ET = {}    # (block_idx, h2) -> list of 16 E tiles
    PSO = {}   # (block_idx, h2) -> psum tile
    dead_sp = [wps]  # S tiles whose exp already ran: fill targets

    def emit_qk_exp(t, jb):
        pr, i0 = BLOCKS[t]
        # interleave the two heads' matmuls: distinct PE row-groups run
        # concurrently (full-array MAC activity keeps HAM at 8/8)
        sps = [psS.tile([128, IS], F32, tag="S", name="sp") for _ in range(2)]
        for nch in range(2):
            for h2 in range(2):
                nc.tensor.matmul(
                    sps[h2][:, ds(nch * 512, 512)],
                    kt_sb[pr][ds(h2 * 64, 64), ds(jb * 128, 128)],
                    qt_sb[pr][ds(h2 * 64, 64), ds(i0 + nch * 512, 512)],
                    start=True, stop=True,
                )
        for h2 in range(2):
            e = etp.tile([128, IS], BF16, tag="et", name="e")
            nc.scalar.activation(e[:], sps[h2][:], AF.Exp, scale=SCALE)
            ET[(t, h2)][jb] = e
            dead_sp.append(sps[h2])

    def emit_pv(t, h2, jbps, pool=None):
        pr, i0 = BLOCKS[t]
        h = pr * 2 + h2
        if (t, h2) not in PSO:
            pool = pool or psO
            tag = "O" if pool is psO else "S"
            PSO[(t, h2)] = pool.tile([128, IS], F32, tag=tag, name="psO")
        O = PSO[(t, h2)]
        for jbp in jbps:
            va = va_sb[jbp // VA_CHUNK]
            vo = (jbp % VA_CHUNK) * H_PER_CORE * VA_W + h * VA_W
            for nch in range(2):
                nc.tensor.matmul(
                    O[0:128, ds(nch * 512, 512)],
                    va[:, ds(vo, VA_W)],
                    ET[(t, h2)][jbp][:, ds(nch * 512, 512)],
                    start=(jbp == 0), stop=(jbp == JB - 1),
                    skip_group_check=True,
                )

    def emit_norm(t, h2, split=False):
        pr, i0 = BLOCKS[t]
        O = PSO[(t, h2)]
        chunks = [(0, 512), (512, 512)] if split else [(0, IS)]
        for off, w in chunks:
            rr = normp.tile([1, w], F32, tag="rr", name="rr")
            nc.vector.reciprocal_approx_fast(rr[:], O[0:1, ds(off, w)])
            bc = normp.tile([128, w], F32, tag="bc", name="bc")
            nc.gpsimd.partition_broadcast(bc[:], rr[0:1, :])
            if h2 == 1:
                # head1's dims are rows 64-127 of at_sb: write in place
                nc.vector.tensor_mul(
                    at_sb[pr][ds(64, 64), ds(i0 + off, w)],
                    O[64:128, ds(off, w)], bc[64:128, :])
            else:
                nm = normp.tile([128, w], BF16, tag="nm", name="nm")
                nc.vector.tensor_mul(nm[64:128, :], O[64:128, ds(off, w)],
                                     bc[64:128, :])
                nc.sync.dma_start(
                    at_sb[pr][ds(0, 64), ds(i0 + off, w)],
                    nm[64:128, :])
        del ET[(t, h2)]

    def emit_proj(ic, eng="vector"):
        # one i-chunk, full 1024 output columns, through a free psS slot
        Y = psS.tile([128, IS], F32, tag="S", name="Ypj")
        for moch in range(2):
            for hd2 in range(2):
                nc.tensor.matmul(
                    Y[:, ds(moch * 512, 512)],
                    at_sb[hd2][:, ds(ic * 128, 128)],
                    wt_sb[hd2][:, ds(moch * 512, 512)],
                    start=(hd2 == 0), stop=(hd2 == 1),
                    skip_group_check=True,
                )
        ysb = youtp.tile([128, IS], BF16, tag="y")
        if eng == "vector":
            nc.vector.tensor_copy(ysb[:], Y[:])
        else:
            nc.scalar.copy(ysb[:], Y[:])
        nc.sync.dma_start(io["y"][ds(ic * 128, 128), :], ysb[:])

    def emit_fill(mac_ns):
        # full-array garbage matmuls (K=128, 512 cols): keep the PE's MAC
        # duty above the HAM re-throttle threshold. Target the previous
        # step's h1 S-tile: its exp is (just) done and its psS slot is not
        # re-allocated until the next step's h0 QK, so the pool's slot-reuse
        # dependency keeps everything ordered.
        tgt = dead_sp[-3] if len(dead_sp) >= 3 else dead_sp[0]
        n = max(0, round(mac_ns / FILL_MAC))
        for i in range(n):
            nc.tensor.matmul(tgt[:, ds(512 * (i % 2), 512)],
                             kt_sb[0][:, 0:128], kt_sb[0][:, 0:512],
                             start=True, stop=True, skip_group_check=True)

    for t in range(len(BLOCKS)):
        for h2 in range(2):
            ET[(t, h2)] = [None] * JB
        for jb in range(JB):
            emit_qk_exp(t, jb)
            pe = QK_PE
            if jb == 1 and t >= 1:
                # previous block's deferred last j-block + its norm
                emit_pv(t - 1, 0, [15])
                emit_norm(t - 1, 0)
                pe += PV_PE
            if t == 0:
                # no previous-block PV: spread own head0 one j-block per step
                if 1 <= jb < JB - 1:
                    emit_pv(0, 0, [jb - 1])
                    pe += PV_PE
                elif jb == JB - 1:
                    emit_pv(0, 0, [14])
                    pe += PV_PE
            else:
                if jb in PV1_SCHED:
                    jbps = PV1_SCHED[jb]
                    emit_pv(t - 1, 1, jbps)
                    pe += PV_PE * len(jbps)
                    if jb == 8:
                        emit_norm(t - 1, 1)
                if jb in PV0_SCHED:
                    jbps = PV0_SCHED[jb]
                    emit_pv(t, 0, jbps)
                    pe += PV_PE * len(jbps)
            emit_fill(PE_TARGET - pe)

    # tail: block 3's head-1 PV accumulates in a freed psS slot (j-blocks
    # 0..13 are ungated and bridge the last exps' retirement), the deferred
    # head0 j-block + norms slot in behind, then the entire projection
    # pipelines through the remaining psS slots
    emit_pv(3, 1, list(range(JB - 2)), pool=psS)
    emit_pv(3, 0, [15])
    emit_norm(3, 0)
    emit_pv(3, 1, [14, 15])
    emit_norm(3, 1, split=True)
    for ic in range(16):
        emit_proj(ic, eng=("scalar" if ic % 2 else "vector"))


_CACHED_NC = None


def _build():
    global _CACHED_NC
    if _CACHED_NC is not None:
        return _CACHED_NC
    nc = bacc.Bacc("TRN2", target_bir_lowering=False, debug=False,
                   num_devices=N_CORES)
    va_cols = JB // VA_CHUNK * H_PER_CORE * VA_W
    io = {
        "qt": nc.dram_tensor("qt", [2, 128, S], BF16,
                             kind="ExternalInput").ap(),
        "kt": nc.dram_tensor("kt", [2, 128, S], BF16,
                             kind="ExternalInput").ap(),
        "va": nc.dram_tensor("va", [VA_CHUNK, 128, va_cols], BF16,
                             kind="ExternalInput").ap(),
        "wt": nc.dram_tensor("wt", [2, 128, D_MODEL], BF16,
                             kind="ExternalInput").ap(),
        "y": nc.dram_tensor("y", [S, D_MODEL], BF16,
                            kind="ExternalOutput").ap(),
        "warm": nc.dram_tensor("warm", [1, 512], F32,
                               kind="ExternalOutput").ap(),
        "wexp": nc.dram_tensor("wexp", [1, 512], F32,
                               kind="ExternalOutput").ap(),
    }
    with tile.TileContext(nc) as tc:
        with ExitStack() as ctx:
            _trace(ctx, tc, io)
    nc.compile()
    _CACHED_NC = nc
    return nc


def _core_inputs(q, k, v, W, b, core):
    bb, g = divmod(core, 4)
    hd0 = g * H_PER_CORE * D_K  # 256 per group
    ncol = H_PER_CORE * D_K
    bf = ml_dtypes.bfloat16

    qt = np.ascontiguousarray(q[bb, :, hd0:hd0 + ncol].T).reshape(2, 128, S)
    kt = np.ascontiguousarray(k[bb, :, hd0:hd0 + ncol].T).reshape(2, 128, S)
    v_sl = v[bb, :, hd0:hd0 + ncol].reshape(S, H_PER_CORE, D_K)
    va = np.concatenate(
        [np.ones((S, H_PER_CORE, 1), np.float32),
         np.zeros((S, H_PER_CORE, 63), np.float32), v_sl], axis=2
    ).reshape(JB, 128, H_PER_CORE * VA_W).transpose(1, 0, 2).reshape(
        128, JB * H_PER_CORE * VA_W)
    va = va.reshape(128, VA_CHUNK, JB // VA_CHUNK * H_PER_CORE * VA_W)
    va = np.ascontiguousarray(va.transpose(1, 0, 2))
    wt = np.ascontiguousarray(W[:, hd0:hd0 + ncol].T).reshape(2, 128, D_MODEL)
    return {
        "qt": qt.astype(bf),
        "kt": kt.astype(bf),
        "va": va.astype(bf),
        "wt": wt.astype(bf),
    }


def run(inputs, trace=False, trace_kwargs=None):
    from concourse.bass_utils import run_bass_kernel_spmd

    q = np.asarray(inputs["q"], np.float32)
    k = np.asarray(inputs["k"], np.float32)
    v = np.asarray(inputs["v"], np.float32)
    W = np.asarray(inputs["W"], np.float32)
    b = np.asarray(inputs["b"], np.float32)

    nc = _build()
    in_maps = [_core_inputs(q, k, v, W, b, c) for c in range(N_CORES)]
    res = run_bass_kernel_spmd(nc, in_maps, core_ids=list(range(N_CORES)),
                               trace=trace, **(trace_kwargs or {}))
    out = np.empty((B, S, D_MODEL), np.float32)
    for bb in range(B):
        acc = res.results[bb * 4 + 0]["y"].astype(np.float32)
        for g in range(1, 4):
            acc = acc + res.results[bb * 4 + g]["y"].astype(np.float32)
        out[bb] = acc + b[None, :]
    return out, res


def kernel(**inputs):
    out, _ = run(inputs)
    return out
